# revision 4
# baseline (speedup 1.0000x reference)
"""Trainium2 Bass kernel for causal multi-head attention (B=4, S=2048, D=1024, H=16).

Sharding: 8 cores = (batch b in 0..3) x (head-group g in 0..1, 8 heads each).
Each core computes, for its (b, g): Q/K/V projections (local 512 dims),
causal attention for 8 heads, and a partial output projection over its 512
head-dims. Host sums the two head-group partials per batch and adds bo.

Structure (v3):
  - q-chunks of 1024: scores for one (head, k-tile) land in a 2-bank
    [128,1024] fp32 PSUM tile -> ONE merged exp per (h, kti) (192 ACTIVATEs),
    minimizing per-instruction overhead on the Scalar/ACT engine (the
    measured bottleneck of the 512-chunk design).
  - heads processed singly (trace shows row-tiled score pairs do not
    overlap on this HW), freeing PSUM: scores 2x2 banks + AV 1x2 +
    projection accumulators 2x1 = exactly 8 banks.
  - K bias dropped: logits shift by (q+bq)*bk, constant along k, which
    softmax cancels exactly. V bias applied on the DVE during the va
    unpack (no PE bias matmul).
  - all operands bf16 (fp32 PSUM accumulation); x-slab DMAs land first so
    the first projection matmul starts ~1.5us in; Scalar queue carries
    DMAs only before the first exp exists.
  - weave: prolog = chunk-0 V,K[t0],Q[t0]; chunk-0 attention weaves the
    remaining chunk-0 + all chunk-1 projections; chunk-1 attention weaves
    oproj m0-7 (chunk-0 rows) for h<4, then t0/t1-partials of m8-15
    (stashed to SBUF) for h>=4; tail = t2/t3 + stash-add only.
  - normalization: ones-column denominator in the A@V stationary operand,
    direct DVE reciprocal of the denominator row, gpsimd
    partition_broadcast, DVE multiply.
"""
import sys
import numpy as np

try:
    import concourse.bass as bass  # noqa: F401
except ImportError:  # pragma: no cover
    sys.path.insert(0, "/opt/trn_rl_repo")

import itertools
from contextlib import ExitStack

import concourse.bacc as bacc
import concourse.tile as tile
import concourse.mybir as mybir
import concourse.bass_utils as bass_utils

B, S, D, H = 4, 2048, 1024, 16
DK = D // H            # 64
G = 2                  # head groups (cores per batch)
HPG = H // G           # 8 heads per core
DG = HPG * DK          # 512 local head dims
NCORES = B * G         # 8
SC = 1024              # q-chunk width
NQC = S // SC          # 2
NKT = S // 128         # 16 k-tiles

F32 = mybir.dt.float32
BF16 = mybir.dt.bfloat16
AF = mybir.ActivationFunctionType

LAST_RESULT = None     # BassKernelResults of the most recent run (for test.py)
_prog = None


def _build():
    nc = bacc.Bacc("TRN2", target_bir_lowering=False, debug=False,
                   num_devices=NCORES)
    xT = nc.dram_tensor("xT", [D, S], BF16, kind="ExternalInput").ap()
    wq = nc.dram_tensor("wq", [D, DG], BF16, kind="ExternalInput").ap()
    wk = nc.dram_tensor("wk", [D, DG], BF16, kind="ExternalInput").ap()
    wv = nc.dram_tensor("wv", [D, DG], BF16, kind="ExternalInput").ap()
    wo = nc.dram_tensor("wo", [DG, D], BF16, kind="ExternalInput").ap()
    bq = nc.dram_tensor("bq", [DG, 1], F32, kind="ExternalInput").ap()
    bv = nc.dram_tensor("bv", [1, DG], F32, kind="ExternalInput").ap()
    vones = nc.dram_tensor("vones", [128, HPG, 1], BF16, kind="ExternalInput").ap()
    tri = nc.dram_tensor("tri", [128, 128], BF16, kind="ExternalInput").ap()
    out = nc.dram_tensor("out", [S, D], F32, kind="ExternalOutput").ap()

    with tile.TileContext(nc) as tc, ExitStack() as ctx:
        cpool = ctx.enter_context(tc.tile_pool(name="consts", bufs=1))
        qkpool = ctx.enter_context(tc.tile_pool(name="qk", bufs=1))
        vpool = ctx.enter_context(tc.tile_pool(name="vaug", bufs=1))
        vtpool = ctx.enter_context(tc.tile_pool(name="vt", bufs=1))
        wpool = ctx.enter_context(tc.tile_pool(name="wqkv", bufs=1))
        xpool = ctx.enter_context(tc.tile_pool(name="xs", bufs=4))
        atpool = ctx.enter_context(tc.tile_pool(name="at", bufs=4))
        pdpool = ctx.enter_context(tc.tile_pool(name="pd", bufs=2))
        scpool = ctx.enter_context(tc.tile_pool(name="scr", bufs=2))
        rpool = ctx.enter_context(tc.tile_pool(name="r0", bufs=2))
        tmpool = ctx.enter_context(tc.tile_pool(name="tmp", bufs=2))
        opool = ctx.enter_context(tc.tile_pool(name="ost", bufs=2))
        stpool = ctx.enter_context(tc.tile_pool(name="stash", bufs=16))
        # PSUM: scores 2x[128,1024] (banks 0-3), AV 1x[128,1024] (4-5),
        # projection/oproj accumulators 2x[128,512] (6-7)
        ppsc = ctx.enter_context(tc.tile_pool(name="ppsc", bufs=2, space="PSUM"))
        ppav = ctx.enter_context(tc.tile_pool(name="ppav", bufs=1, space="PSUM"))
        ppacc = ctx.enter_context(tc.tile_pool(name="ppacc", bufs=2, space="PSUM"))

        xTr = xT.rearrange("(c p) s -> p c s", p=128)
        wqr = wq.rearrange("(c p) n -> p c n", p=128)
        wkr = wk.rearrange("(c p) n -> p c n", p=128)
        wvr = wv.rearrange("(c p) n -> p c n", p=128)

        # startup: all x slabs first (the V prolog contracts over all 8),
        # wv in parallel on gpsimd, then wq/wk. Scalar-queue DMAs are
        # startup-only (ACT idles until the first exp ~35us in).
        xs0_0 = xpool.tile([128, 4, SC], BF16, tag="xs", name="xs0_0")
        xs1_0 = xpool.tile([128, 4, SC], BF16, tag="xs", name="xs1_0")
        wq_t = wpool.tile([128, 8, DG], BF16)
        wk_t = wpool.tile([128, 8, DG], BF16)
        wv_t = wpool.tile([128, 8, DG], BF16)
        for c in range(4):
            nc.sync.dma_start(xs0_0[:, c, :], xTr[:, c, 0:SC])
            nc.scalar.dma_start(xs1_0[:, c, :], xTr[:, 4 + c, 0:SC])
        for c in range(8):
            nc.gpsimd.dma_start(wv_t[:, c, :], wvr[:, c, :])
        for c in range(4):
            nc.sync.dma_start(wq_t[:, c, :], wqr[:, c, :])
            nc.scalar.dma_start(wq_t[:, 4 + c, :], wqr[:, 4 + c, :])
            nc.sync.dma_start(wk_t[:, c, :], wkr[:, c, :])
            nc.scalar.dma_start(wk_t[:, 4 + c, :], wkr[:, 4 + c, :])

        bq_t = cpool.tile([128, 4], F32)
        nc.gpsimd.dma_start(bq_t[:], bq.rearrange("(t p) o -> p (t o)", p=128))
        bv_t = cpool.tile([1, DG], F32)
        nc.gpsimd.dma_start(bv_t[:], bv)
        bvfull = cpool.tile([128, DG], F32)
        nc.gpsimd.partition_broadcast(bvfull[:], bv_t[:])
        tri_t = cpool.tile([128, 128], BF16)
        nc.gpsimd.dma_start(tri_t[:], tri)

        qt = [qkpool.tile([128, S], BF16, name=f"qt{t}") for t in range(4)]
        kt_ = [qkpool.tile([128, S], BF16, name=f"kt{t}") for t in range(4)]
        vt = [vtpool.tile([128, S], BF16, name=f"vt{t}") for t in range(4)]
        va = [vpool.tile([128, HPG * 65], BF16, name=f"va{i}") for i in range(NKT)]
        for i in range(NKT):
            nc.gpsimd.dma_start(
                va[i].rearrange("p (h c) -> p h c", c=65)[:, :, 64:65], vones)
        # wo only needed by the output projection (chunk-1 filler onward)
        wo_t = cpool.tile([128, 4, D], BF16)
        nc.gpsimd.dma_start(wo_t[:], wo.rearrange("(c p) n -> p c n", p=128))

        def qgroup(sc, t, half, xc):
            pq = ppacc.tile([128, 512], F32, tag="pacc", name=f"pq{sc}_{t}_{half}")
            for c in range(8):
                nc.tensor.matmul(pq[:], wq_t[:, c, t * 128:(t + 1) * 128],
                                 xc(c)[:, half * 512:(half + 1) * 512],
                                 start=(c == 0), stop=(c == 7))
                yield
            lo = sc * SC + half * 512
            nc.vector.tensor_scalar_add(qt[t][:, lo:lo + 512], pq[:],
                                        bq_t[:, t:t + 1])

        def kgroup(sc, t, half, xc):
            pk = ppacc.tile([128, 512], F32, tag="pacc", name=f"pk{sc}_{t}_{half}")
            for c in range(8):
                nc.tensor.matmul(pk[:], wk_t[:, c, t * 128:(t + 1) * 128],
                                 xc(c)[:, half * 512:(half + 1) * 512],
                                 start=(c == 0), stop=(c == 7))
                yield
            lo = sc * SC + half * 512
            # no K bias: it shifts logits by a per-q constant along k,
            # which softmax cancels exactly
            nc.vector.tensor_copy(kt_[t][:, lo:lo + 512], pk[:])

        def vgroup(sc, ms, xc):
            pv = ppacc.tile([128, 512], F32, tag="pacc", name=f"pv{sc}_{ms}")
            for c in range(8):
                nc.tensor.matmul(pv[:], xc(c)[:, ms * 128:(ms + 1) * 128],
                                 wv_t[:, c, :], start=(c == 0), stop=(c == 7))
                yield
            i = 8 * sc + ms
            nc.vector.tensor_add(
                va[i].rearrange("p (h c) -> p h c", c=65)[:, :, 0:64],
                pv[:].rearrange("p (h c) -> p h c", c=64),
                bvfull[:].rearrange("p (h c) -> p h c", c=64))
            yield

        def make_xc(sc, slabs=None):
            if slabs is not None:
                xs0, xs1 = slabs
            else:
                xs0 = xpool.tile([128, 4, SC], BF16, tag="xs", name=f"xs0_{sc}")
                xs1 = xpool.tile([128, 4, SC], BF16, tag="xs", name=f"xs1_{sc}")
                for c in range(4):
                    nc.sync.dma_start(xs0[:, c, :],
                                      xTr[:, c, sc * SC:(sc + 1) * SC])
                    nc.gpsimd.dma_start(xs1[:, c, :],
                                        xTr[:, 4 + c, sc * SC:(sc + 1) * SC])
            return lambda c: (xs0 if c < 4 else xs1)[:, c % 4, :]

        def proj_rest_gen(sc, xc):
            # Q/K for t=1..3 (t=0 and V are done in the prolog for sc=0)
            for t in range(1, 4):
                for half in range(2):
                    yield from kgroup(sc, t, half, xc)
                for half in range(2):
                    yield from qgroup(sc, t, half, xc)

        def proj_full_gen(sc):
            xc = make_xc(sc)
            for ms in range(8):
                yield from vgroup(sc, ms, xc)
            for half in range(2):
                yield from kgroup(sc, 0, half, xc)
            for half in range(2):
                yield from qgroup(sc, 0, half, xc)
            yield from proj_rest_gen(sc, xc)

        def oproj_g(m):
            ot = opool.tile([128, D], F32, tag="ost", name=f"ot{m}")
            for nh in range(2):
                pon = ppacc.tile([128, 512], F32, tag="pacc", name=f"pon{m}_{nh}")
                for t in range(4):
                    nc.tensor.matmul(pon[:], vt[t][:, m * 128:(m + 1) * 128],
                                     wo_t[:, t, nh * 512:(nh + 1) * 512],
                                     start=(t == 0), stop=(t == 3))
                    yield
                nc.vector.tensor_copy(ot[:, nh * 512:(nh + 1) * 512], pon[:])
                yield
            nc.sync.dma_start(out[m * 128:(m + 1) * 128, :], ot[:])

        stash = {}

        def oproj_partial_gen():
            # t0/t1 partials for the chunk-1 rows; safe once vt[0], vt[1]
            # chunk-1 columns exist (after head 3) — pulled only for h>=4
            for m in range(8, 16):
                for nh in range(2):
                    pon = ppacc.tile([128, 512], F32, tag="pacc",
                                     name=f"pp{m}_{nh}")
                    for t in range(2):
                        nc.tensor.matmul(pon[:], vt[t][:, m * 128:(m + 1) * 128],
                                         wo_t[:, t, nh * 512:(nh + 1) * 512],
                                         start=(t == 0), stop=(t == 1))
                        yield
                    st = stpool.tile([128, 512], BF16, tag="st",
                                     name=f"st{m}_{nh}")
                    nc.vector.tensor_copy(st[:], pon[:])
                    stash[(m, nh)] = st
                    yield

        def oproj_tail_g(m):
            ot = opool.tile([128, D], F32, tag="ost", name=f"ot{m}")
            for nh in range(2):
                pon = ppacc.tile([128, 512], F32, tag="pacc", name=f"pt{m}_{nh}")
                for t in range(2, 4):
                    nc.tensor.matmul(pon[:], vt[t][:, m * 128:(m + 1) * 128],
                                     wo_t[:, t, nh * 512:(nh + 1) * 512],
                                     start=(t == 2), stop=(t == 3))
                nc.vector.tensor_add(ot[:, nh * 512:(nh + 1) * 512], pon[:],
                                     stash[(m, nh)][:])
            nc.sync.dma_start(out[m * 128:(m + 1) * 128, :], ot[:])

        def attn_head(h, qc, pull):
            t, po = h // 2, 64 * (h % 2)
            nkt = 8 * qc + 8
            pav = ppav.tile([128, SC], F32, tag="pav", name=f"pav{h}_{qc}")
            for kti in range(nkt):
                delta = max(kti * 128 - qc * SC, 0)
                pss = ppsc.tile([128, SC], F32, tag="ps", name=f"ps{h}_{qc}_{kti}")
                ks = kt_[t][po:po + 64, kti * 128:(kti + 1) * 128]
                q0 = qc * SC
                if delta < 512:
                    nc.tensor.matmul(pss[:, delta:512], ks,
                                     qt[t][po:po + 64, q0 + delta:q0 + 512],
                                     start=True, stop=True)
                h1lo = max(delta, 512)
                nc.tensor.matmul(pss[:, h1lo:SC], ks,
                                 qt[t][po:po + 64, q0 + h1lo:q0 + SC],
                                 start=True, stop=True)
                at = atpool.tile([128, SC], BF16, tag="at", name=f"at{h}_{qc}_{kti}")
                nc.scalar.activation(at[:, delta:SC], pss[:, delta:SC],
                                     AF.Exp, scale=0.125)
                if kti >= 8 * qc:  # diagonal 128x128 square of the band tile
                    sl = slice(delta, delta + 128)
                    nc.vector.tensor_mul(at[:, sl], at[:, sl], tri_t[:])
                vs = va[kti][:, h * 65:(h + 1) * 65]
                if delta < 512:
                    nc.tensor.matmul(pav[0:65, delta:512], vs, at[:, delta:512],
                                     start=(kti == 0), stop=(kti == 8 * qc + 3))
                nc.tensor.matmul(pav[0:65, h1lo:SC], vs, at[:, h1lo:SC],
                                 start=(kti == 0), stop=(kti == nkt - 1))
                pull(4 if qc == 0 else 2)
            # drain + normalize: denominator sits in row 64 of pav
            pd = pdpool.tile([128, SC], F32, tag="pd", name=f"pd{h}_{qc}")
            nc.vector.tensor_copy(pd[0:65, :], pav[0:65, :])
            r0 = rpool.tile([1, SC], F32, tag="r0", name=f"r0{h}_{qc}")
            nc.vector.reciprocal(r0[:], pd[64:65, :])
            sct = scpool.tile([128, SC], F32, tag="scr", name=f"sc{h}_{qc}")
            nc.gpsimd.partition_broadcast(sct[0:64, :], r0[:])
            if po == 0:
                nc.vector.tensor_mul(vt[t][0:64, qc * SC:(qc + 1) * SC],
                                     pd[0:64, :], sct[0:64, :])
            else:
                tmp = tmpool.tile([64, SC], BF16, tag="tmp", name=f"tm{h}_{qc}")
                nc.vector.tensor_mul(tmp[:], pd[0:64, :], sct[0:64, :])
                nc.sync.dma_start(vt[t][64:128, qc * SC:(qc + 1) * SC], tmp[:])
            pull(2)

        # ---- prolog: chunk-0 V, K[t0], Q[t0] (minimum for attention h=0)
        xc0 = make_xc(0, slabs=(xs0_0, xs1_0))
        for ms in range(8):
            for _ in vgroup(0, ms, xc0):
                pass
        for half in range(2):
            for _ in kgroup(0, 0, half, xc0):
                pass
        for half in range(2):
            for _ in qgroup(0, 0, half, xc0):
                pass

        # ---- chunk 0: attention, weaving the rest of proj-0 then proj-1
        filler = itertools.chain(proj_rest_gen(0, xc0), proj_full_gen(1))

        def pull(n):
            for _ in range(n):
                if next(filler, "END") == "END":
                    return

        for h in range(HPG):
            attn_head(h, 0, pull)
        for _ in filler:   # any proj-1 leftovers before chunk-1 attention
            pass

        # ---- chunk 1: attention; filler = oproj m0-7, then (h>=4) the
        # t0/t1 partials of m8-15
        filler = oproj_gen = (y for m in range(8) for y in oproj_g(m))
        for h in range(HPG):
            if h == 4:
                filler = itertools.chain(filler, oproj_partial_gen())

                def pull(n, _f=filler):
                    for _ in range(n):
                        if next(_f, "END") == "END":
                            return
            attn_head(h, 1, pull)
        for _ in filler:
            pass
        # ---- tail: finish m8-15 with t2/t3 + stash add
        for m in range(8, 16):
            oproj_tail_g(m)

    nc.compile()
    return nc


def _program():
    global _prog
    if _prog is None:
        _prog = _build()
    return _prog


def kernel(x, mask, Wq, bq, Wk, bk, Wv, bv, Wo, bo):
    global LAST_RESULT
    import ml_dtypes
    x = np.asarray(x, dtype=np.float32)
    Wq = np.asarray(Wq, dtype=np.float32)
    Wk = np.asarray(Wk, dtype=np.float32)
    Wv = np.asarray(Wv, dtype=np.float32)
    Wo = np.asarray(Wo, dtype=np.float32)
    bq = np.asarray(bq, dtype=np.float32)
    bv = np.asarray(bv, dtype=np.float32)
    bo = np.asarray(bo, dtype=np.float32)

    nc = _program()
    bf = ml_dtypes.bfloat16
    xTs = [np.ascontiguousarray(x[b].T).astype(bf) for b in range(B)]
    tri = np.zeros((128, 128), dtype=bf)
    tri[np.triu_indices(128)] = 1.0
    in_maps = []
    for c in range(NCORES):
        b, g = divmod(c, G)
        sl = slice(g * DG, (g + 1) * DG)
        in_maps.append({
            "xT": xTs[b],
            "wq": np.ascontiguousarray(Wq[sl, :].T).astype(bf),
            "wk": np.ascontiguousarray(Wk[sl, :].T).astype(bf),
            "wv": np.ascontiguousarray(Wv[sl, :].T).astype(bf),
            "wo": np.ascontiguousarray(Wo[:, sl].T).astype(bf),
            "bq": np.ascontiguousarray(bq[sl].reshape(DG, 1)),
            "bv": np.ascontiguousarray(bv[sl].reshape(1, DG)),
            "vones": np.ones((128, HPG, 1), dtype=bf),
            "tri": tri,
        })
    res = bass_utils.run_bass_kernel_spmd(nc, in_maps,
                                          core_ids=list(range(NCORES)))
    LAST_RESULT = res
    outs = [r["out"] for r in res.results]
    y = np.stack([outs[G * b] + outs[G * b + 1] for b in range(B)])
    y += bo[None, None, :]
    return y.astype(np.float32)


# revision 5
# speedup vs baseline: 1.3130x; 1.3130x over previous
"""Trainium2 Bass kernel for causal multi-head attention (B=4, S=2048, D=1024, H=16).

Sharding: 8 cores = (batch b in 0..3) x (head-group g in 0..1, 8 heads each).
Each core computes, for its (b, g): Q/K/V projections (local 512 dims),
causal attention for 8 heads, and a partial output projection over its 512
head-dims. Host sums the two head-group partials per batch and adds bo.

Structure (v3):
  - q-chunks of 1024: scores for one (head, k-tile) land in a 2-bank
    [128,1024] fp32 PSUM tile -> ONE merged exp per (h, kti) (192 ACTIVATEs),
    minimizing per-instruction overhead on the Scalar/ACT engine (the
    measured bottleneck of the 512-chunk design).
  - heads processed singly (trace shows row-tiled score pairs do not
    overlap on this HW), freeing PSUM: scores 2x2 banks + AV 1x2 +
    projection accumulators 2x1 = exactly 8 banks.
  - K bias dropped: logits shift by (q+bq)*bk, constant along k, which
    softmax cancels exactly. V bias applied on the DVE during the va
    unpack (no PE bias matmul).
  - all operands bf16 (fp32 PSUM accumulation); x-slab DMAs land first so
    the first projection matmul starts ~1.5us in; Scalar queue carries
    DMAs only before the first exp exists.
  - weave: prolog = chunk-0 V,K[t0],Q[t0]; chunk-0 attention weaves the
    remaining chunk-0 + all chunk-1 projections; chunk-1 attention weaves
    oproj m0-7 (chunk-0 rows) for h<4, then t0/t1-partials of m8-15
    (stashed to SBUF) for h>=4; tail = t2/t3 + stash-add only.
  - normalization: ones-column denominator in the A@V stationary operand,
    direct DVE reciprocal of the denominator row, gpsimd
    partition_broadcast, DVE multiply.
"""
import sys
import numpy as np

try:
    import concourse.bass as bass  # noqa: F401
except ImportError:  # pragma: no cover
    sys.path.insert(0, "/opt/trn_rl_repo")

import itertools
from contextlib import ExitStack

import concourse.bacc as bacc
import concourse.tile as tile
import concourse.mybir as mybir
import concourse.bass_utils as bass_utils

B, S, D, H = 4, 2048, 1024, 16
DK = D // H            # 64
G = 2                  # head groups (cores per batch)
HPG = H // G           # 8 heads per core
DG = HPG * DK          # 512 local head dims
NCORES = B * G         # 8
SC = 1024              # q-chunk width
NQC = S // SC          # 2
NKT = S // 128         # 16 k-tiles

F32 = mybir.dt.float32
BF16 = mybir.dt.bfloat16
AF = mybir.ActivationFunctionType

LAST_RESULT = None     # BassKernelResults of the most recent run (for test.py)
_prog = None


def _build():
    nc = bacc.Bacc("TRN2", target_bir_lowering=False, debug=False,
                   num_devices=NCORES)
    xT = nc.dram_tensor("xT", [D, S], BF16, kind="ExternalInput").ap()
    wq = nc.dram_tensor("wq", [D, DG], BF16, kind="ExternalInput").ap()
    wk = nc.dram_tensor("wk", [D, DG], BF16, kind="ExternalInput").ap()
    wv = nc.dram_tensor("wv", [D, DG], BF16, kind="ExternalInput").ap()
    wo = nc.dram_tensor("wo", [DG, D], BF16, kind="ExternalInput").ap()
    bq = nc.dram_tensor("bq", [DG, 1], F32, kind="ExternalInput").ap()
    bv = nc.dram_tensor("bv", [1, DG], F32, kind="ExternalInput").ap()
    vones = nc.dram_tensor("vones", [128, HPG, 1], BF16, kind="ExternalInput").ap()
    tri = nc.dram_tensor("tri", [128, 128], BF16, kind="ExternalInput").ap()
    out = nc.dram_tensor("out", [S, D], F32, kind="ExternalOutput").ap()

    with tile.TileContext(nc) as tc, ExitStack() as ctx:
        cpool = ctx.enter_context(tc.tile_pool(name="consts", bufs=1))
        qkpool = ctx.enter_context(tc.tile_pool(name="qk", bufs=1))
        vpool = ctx.enter_context(tc.tile_pool(name="vaug", bufs=1))
        vtpool = ctx.enter_context(tc.tile_pool(name="vt", bufs=1))
        wpool = ctx.enter_context(tc.tile_pool(name="wqkv", bufs=1))
        xpool = ctx.enter_context(tc.tile_pool(name="xs", bufs=4))
        atpool = ctx.enter_context(tc.tile_pool(name="at", bufs=4))
        pdpool = ctx.enter_context(tc.tile_pool(name="pd", bufs=2))
        scpool = ctx.enter_context(tc.tile_pool(name="scr", bufs=2))
        rpool = ctx.enter_context(tc.tile_pool(name="r0", bufs=2))
        tmpool = ctx.enter_context(tc.tile_pool(name="tmp", bufs=2))
        opool = ctx.enter_context(tc.tile_pool(name="ost", bufs=2))
        stpool = ctx.enter_context(tc.tile_pool(name="stash", bufs=16))
        # PSUM: scores 2x[128,1024] (banks 0-3), AV 1x[128,1024] (4-5),
        # projection/oproj accumulators 2x[128,512] (6-7)
        ppsc = ctx.enter_context(tc.tile_pool(name="ppsc", bufs=2, space="PSUM"))
        ppav = ctx.enter_context(tc.tile_pool(name="ppav", bufs=1, space="PSUM"))
        ppacc = ctx.enter_context(tc.tile_pool(name="ppacc", bufs=2, space="PSUM"))

        xTr = xT.rearrange("(c p) s -> p c s", p=128)
        wqr = wq.rearrange("(c p) n -> p c n", p=128)
        wkr = wk.rearrange("(c p) n -> p c n", p=128)
        wvr = wv.rearrange("(c p) n -> p c n", p=128)

        # startup: all x slabs first (the V prolog contracts over all 8),
        # wv in parallel on gpsimd, then wq/wk. Scalar-queue DMAs are
        # startup-only (ACT idles until the first exp ~35us in).
        xs0_0 = xpool.tile([128, 4, SC], BF16, tag="xs", name="xs0_0")
        xs1_0 = xpool.tile([128, 4, SC], BF16, tag="xs", name="xs1_0")
        wq_t = wpool.tile([128, 8, DG], BF16)
        wk_t = wpool.tile([128, 8, DG], BF16)
        wv_t = wpool.tile([128, 8, DG], BF16)
        for c in range(4):
            nc.sync.dma_start(xs0_0[:, c, :], xTr[:, c, 0:SC])
            nc.scalar.dma_start(xs1_0[:, c, :], xTr[:, 4 + c, 0:SC])
        for c in range(8):
            nc.gpsimd.dma_start(wv_t[:, c, :], wvr[:, c, :])
        for c in range(4):
            nc.sync.dma_start(wq_t[:, c, :], wqr[:, c, :])
            nc.scalar.dma_start(wq_t[:, 4 + c, :], wqr[:, 4 + c, :])
            nc.sync.dma_start(wk_t[:, c, :], wkr[:, c, :])
            nc.scalar.dma_start(wk_t[:, 4 + c, :], wkr[:, 4 + c, :])

        bq_t = cpool.tile([128, 4], F32)
        nc.gpsimd.dma_start(bq_t[:], bq.rearrange("(t p) o -> p (t o)", p=128))
        bv_t = cpool.tile([1, DG], F32)
        nc.gpsimd.dma_start(bv_t[:], bv)
        bvfull = cpool.tile([128, DG], F32)
        nc.gpsimd.partition_broadcast(bvfull[:], bv_t[:])
        tri_t = cpool.tile([128, 128], BF16)
        nc.gpsimd.dma_start(tri_t[:], tri)

        qt = [qkpool.tile([128, S], BF16, name=f"qt{t}") for t in range(4)]
        kt_ = [qkpool.tile([128, S], BF16, name=f"kt{t}") for t in range(4)]
        vt = [vtpool.tile([128, S], BF16, name=f"vt{t}") for t in range(4)]
        va = [vpool.tile([128, HPG * 65], BF16, name=f"va{i}") for i in range(NKT)]
        for i in range(NKT):
            nc.gpsimd.dma_start(
                va[i].rearrange("p (h c) -> p h c", c=65)[:, :, 64:65], vones)
        # wo only needed by the output projection (chunk-1 filler onward)
        wo_t = cpool.tile([128, 4, D], BF16)
        nc.gpsimd.dma_start(wo_t[:], wo.rearrange("(c p) n -> p c n", p=128))

        def qgroup(sc, t, half, xc):
            pq = ppacc.tile([128, 512], F32, tag="pacc", name=f"pq{sc}_{t}_{half}")
            for c in range(8):
                nc.tensor.matmul(pq[:], wq_t[:, c, t * 128:(t + 1) * 128],
                                 xc(c)[:, half * 512:(half + 1) * 512],
                                 start=(c == 0), stop=(c == 7))
                yield
            lo = sc * SC + half * 512
            nc.vector.tensor_scalar_add(qt[t][:, lo:lo + 512], pq[:],
                                        bq_t[:, t:t + 1])

        def kgroup(sc, t, half, xc):
            pk = ppacc.tile([128, 512], F32, tag="pacc", name=f"pk{sc}_{t}_{half}")
            for c in range(8):
                nc.tensor.matmul(pk[:], wk_t[:, c, t * 128:(t + 1) * 128],
                                 xc(c)[:, half * 512:(half + 1) * 512],
                                 start=(c == 0), stop=(c == 7))
                yield
            lo = sc * SC + half * 512
            # no K bias: it shifts logits by a per-q constant along k,
            # which softmax cancels exactly
            nc.vector.tensor_copy(kt_[t][:, lo:lo + 512], pk[:])

        def vgroup(sc, ms, xc):
            pv = ppacc.tile([128, 512], F32, tag="pacc", name=f"pv{sc}_{ms}")
            for c in range(8):
                nc.tensor.matmul(pv[:], xc(c)[:, ms * 128:(ms + 1) * 128],
                                 wv_t[:, c, :], start=(c == 0), stop=(c == 7))
                yield
            i = 8 * sc + ms
            nc.vector.tensor_add(
                va[i].rearrange("p (h c) -> p h c", c=65)[:, :, 0:64],
                pv[:].rearrange("p (h c) -> p h c", c=64),
                bvfull[:].rearrange("p (h c) -> p h c", c=64))
            yield

        def make_xc(sc, slabs=None):
            if slabs is not None:
                xs0, xs1 = slabs
            else:
                xs0 = xpool.tile([128, 4, SC], BF16, tag="xs", name=f"xs0_{sc}")
                xs1 = xpool.tile([128, 4, SC], BF16, tag="xs", name=f"xs1_{sc}")
                for c in range(4):
                    nc.sync.dma_start(xs0[:, c, :],
                                      xTr[:, c, sc * SC:(sc + 1) * SC])
                    nc.gpsimd.dma_start(xs1[:, c, :],
                                        xTr[:, 4 + c, sc * SC:(sc + 1) * SC])
            return lambda c: (xs0 if c < 4 else xs1)[:, c % 4, :]

        def proj_rest_gen(sc, xc):
            # Q/K for t=1..3 (t=0 and V are done in the prolog for sc=0)
            for t in range(1, 4):
                for half in range(2):
                    yield from kgroup(sc, t, half, xc)
                for half in range(2):
                    yield from qgroup(sc, t, half, xc)

        def proj_full_gen(sc):
            xc = make_xc(sc)
            for ms in range(8):
                yield from vgroup(sc, ms, xc)
            for half in range(2):
                yield from kgroup(sc, 0, half, xc)
            for half in range(2):
                yield from qgroup(sc, 0, half, xc)
            yield from proj_rest_gen(sc, xc)

        def oproj_g(m):
            ot = opool.tile([128, D], F32, tag="ost", name=f"ot{m}")
            for nh in range(2):
                pon = ppacc.tile([128, 512], F32, tag="pacc", name=f"pon{m}_{nh}")
                for t in range(4):
                    nc.tensor.matmul(pon[:], vt[t][:, m * 128:(m + 1) * 128],
                                     wo_t[:, t, nh * 512:(nh + 1) * 512],
                                     start=(t == 0), stop=(t == 3))
                    yield
                nc.vector.tensor_copy(ot[:, nh * 512:(nh + 1) * 512], pon[:])
                yield
            nc.sync.dma_start(out[m * 128:(m + 1) * 128, :], ot[:])

        stash = {}

        def oproj_partial_gen():
            # t0/t1 partials for the chunk-1 rows; safe once vt[0], vt[1]
            # chunk-1 columns exist (after head 3) — pulled only for h>=4
            for m in range(8, 16):
                for nh in range(2):
                    pon = ppacc.tile([128, 512], F32, tag="pacc",
                                     name=f"pp{m}_{nh}")
                    for t in range(2):
                        nc.tensor.matmul(pon[:], vt[t][:, m * 128:(m + 1) * 128],
                                         wo_t[:, t, nh * 512:(nh + 1) * 512],
                                         start=(t == 0), stop=(t == 1))
                        yield
                    st = stpool.tile([128, 512], BF16, tag="st",
                                     name=f"st{m}_{nh}")
                    nc.vector.tensor_copy(st[:], pon[:])
                    stash[(m, nh)] = st
                    yield

        def oproj_tail_g(m):
            ot = opool.tile([128, D], F32, tag="ost", name=f"ot{m}")
            for nh in range(2):
                pon = ppacc.tile([128, 512], F32, tag="pacc", name=f"pt{m}_{nh}")
                for t in range(2, 4):
                    nc.tensor.matmul(pon[:], vt[t][:, m * 128:(m + 1) * 128],
                                     wo_t[:, t, nh * 512:(nh + 1) * 512],
                                     start=(t == 2), stop=(t == 3))
                nc.vector.tensor_add(ot[:, nh * 512:(nh + 1) * 512], pon[:],
                                     stash[(m, nh)][:])
            nc.sync.dma_start(out[m * 128:(m + 1) * 128, :], ot[:])

        def attn_head(h, qc, pull):
            t, po = h // 2, 64 * (h % 2)
            nkt = 8 * qc + 8
            pav = ppav.tile([128, SC], F32, tag="pav", name=f"pav{h}_{qc}")
            for kti in range(nkt):
                delta = max(kti * 128 - qc * SC, 0)
                pss = ppsc.tile([128, SC], F32, tag="ps", name=f"ps{h}_{qc}_{kti}")
                ks = kt_[t][po:po + 64, kti * 128:(kti + 1) * 128]
                q0 = qc * SC
                if delta < 512:
                    nc.tensor.matmul(pss[:, delta:512], ks,
                                     qt[t][po:po + 64, q0 + delta:q0 + 512],
                                     start=True, stop=True)
                h1lo = max(delta, 512)
                nc.tensor.matmul(pss[:, h1lo:SC], ks,
                                 qt[t][po:po + 64, q0 + h1lo:q0 + SC],
                                 start=True, stop=True)
                at = atpool.tile([128, SC], BF16, tag="at", name=f"at{h}_{qc}_{kti}")
                nc.scalar.activation(at[:, delta:SC], pss[:, delta:SC],
                                     AF.Exp, scale=0.125)
                if kti >= 8 * qc:  # diagonal 128x128 square of the band tile
                    sl = slice(delta, delta + 128)
                    nc.vector.tensor_mul(at[:, sl], at[:, sl], tri_t[:])
                vs = va[kti][:, h * 65:(h + 1) * 65]
                if delta < 512:
                    nc.tensor.matmul(pav[0:65, delta:512], vs, at[:, delta:512],
                                     start=(kti == 0), stop=(kti == 8 * qc + 3))
                nc.tensor.matmul(pav[0:65, h1lo:SC], vs, at[:, h1lo:SC],
                                 start=(kti == 0), stop=(kti == nkt - 1))
                pull(4 if qc == 0 else 2)
            # drain + normalize: denominator sits in row 64 of pav
            pd = pdpool.tile([128, SC], F32, tag="pd", name=f"pd{h}_{qc}")
            nc.vector.tensor_copy(pd[0:65, :], pav[0:65, :])
            # spread the 1024 denominators over 128 partitions: the exact
            # reciprocal runs at 8 elements/lane instead of 1024 on one lane
            rb = rpool.tile([128, 8], F32, tag="rb", name=f"rb{h}_{qc}")
            nc.gpsimd.dma_start(rb[:], pd[64:65, :])
            nc.vector.reciprocal(rb[:], rb[:])
            r0 = rpool.tile([1, SC], F32, tag="r0", name=f"r0{h}_{qc}")
            nc.gpsimd.dma_start(r0[:], rb[:])
            sct = scpool.tile([128, SC], F32, tag="scr", name=f"sc{h}_{qc}")
            nc.gpsimd.partition_broadcast(sct[0:64, :], r0[:])
            if po == 0:
                nc.vector.tensor_mul(vt[t][0:64, qc * SC:(qc + 1) * SC],
                                     pd[0:64, :], sct[0:64, :])
            else:
                tmp = tmpool.tile([64, SC], BF16, tag="tmp", name=f"tm{h}_{qc}")
                nc.vector.tensor_mul(tmp[:], pd[0:64, :], sct[0:64, :])
                nc.sync.dma_start(vt[t][64:128, qc * SC:(qc + 1) * SC], tmp[:])
            pull(2)

        # ---- prolog: chunk-0 V, K[t0], Q[t0] (minimum for attention h=0)
        xc0 = make_xc(0, slabs=(xs0_0, xs1_0))
        for ms in range(8):
            for _ in vgroup(0, ms, xc0):
                pass
        for half in range(2):
            for _ in kgroup(0, 0, half, xc0):
                pass
        for half in range(2):
            for _ in qgroup(0, 0, half, xc0):
                pass

        # ---- chunk 0: attention, weaving the rest of proj-0 then proj-1
        filler = itertools.chain(proj_rest_gen(0, xc0), proj_full_gen(1))

        def pull(n):
            for _ in range(n):
                if next(filler, "END") == "END":
                    return

        for h in range(HPG):
            attn_head(h, 0, pull)
        for _ in filler:   # any proj-1 leftovers before chunk-1 attention
            pass

        # ---- chunk 1: attention; filler = oproj m0-7, then (h>=4) the
        # t0/t1 partials of m8-15
        filler = oproj_gen = (y for m in range(8) for y in oproj_g(m))
        for h in range(HPG):
            if h == 4:
                filler = itertools.chain(filler, oproj_partial_gen())

                def pull(n, _f=filler):
                    for _ in range(n):
                        if next(_f, "END") == "END":
                            return
            attn_head(h, 1, pull)
        for _ in filler:
            pass
        # ---- tail: finish m8-15 with t2/t3 + stash add
        for m in range(8, 16):
            oproj_tail_g(m)

    nc.compile()
    return nc


def _program():
    global _prog
    if _prog is None:
        _prog = _build()
    return _prog


def kernel(x, mask, Wq, bq, Wk, bk, Wv, bv, Wo, bo):
    global LAST_RESULT
    import ml_dtypes
    x = np.asarray(x, dtype=np.float32)
    Wq = np.asarray(Wq, dtype=np.float32)
    Wk = np.asarray(Wk, dtype=np.float32)
    Wv = np.asarray(Wv, dtype=np.float32)
    Wo = np.asarray(Wo, dtype=np.float32)
    bq = np.asarray(bq, dtype=np.float32)
    bv = np.asarray(bv, dtype=np.float32)
    bo = np.asarray(bo, dtype=np.float32)

    nc = _program()
    bf = ml_dtypes.bfloat16
    xTs = [np.ascontiguousarray(x[b].T).astype(bf) for b in range(B)]
    tri = np.zeros((128, 128), dtype=bf)
    tri[np.triu_indices(128)] = 1.0
    in_maps = []
    for c in range(NCORES):
        b, g = divmod(c, G)
        sl = slice(g * DG, (g + 1) * DG)
        in_maps.append({
            "xT": xTs[b],
            "wq": np.ascontiguousarray(Wq[sl, :].T).astype(bf),
            "wk": np.ascontiguousarray(Wk[sl, :].T).astype(bf),
            "wv": np.ascontiguousarray(Wv[sl, :].T).astype(bf),
            "wo": np.ascontiguousarray(Wo[:, sl].T).astype(bf),
            "bq": np.ascontiguousarray(bq[sl].reshape(DG, 1)),
            "bv": np.ascontiguousarray(bv[sl].reshape(1, DG)),
            "vones": np.ones((128, HPG, 1), dtype=bf),
            "tri": tri,
        })
    res = bass_utils.run_bass_kernel_spmd(nc, in_maps,
                                          core_ids=list(range(NCORES)))
    LAST_RESULT = res
    outs = [r["out"] for r in res.results]
    y = np.stack([outs[G * b] + outs[G * b + 1] for b in range(B)])
    y += bo[None, None, :]
    return y.astype(np.float32)


# revision 11
# speedup vs baseline: 1.3257x; 1.0096x over previous
"""Trainium2 Bass kernel for causal multi-head attention (B=4, S=2048, D=1024, H=16).

Sharding: 8 cores = (batch b in 0..3) x (head-group g in 0..1, 8 heads each).
Each core computes, for its (b, g): Q/K/V projections (local 512 dims),
causal attention for 8 heads, and a partial output projection over its 512
head-dims. Host sums the two head-group partials per batch and adds bo.

Structure (v3):
  - q-chunks of 1024: scores for one (head, k-tile) land in a 2-bank
    [128,1024] fp32 PSUM tile -> ONE merged exp per (h, kti) (192 ACTIVATEs),
    minimizing per-instruction overhead on the Scalar/ACT engine (the
    measured bottleneck of the 512-chunk design).
  - heads processed singly (trace shows row-tiled score pairs do not
    overlap on this HW), freeing PSUM: scores 2x2 banks + AV 1x2 +
    projection accumulators 2x1 = exactly 8 banks.
  - K bias dropped: logits shift by (q+bq)*bk, constant along k, which
    softmax cancels exactly. V bias applied on the DVE during the va
    unpack (no PE bias matmul).
  - all operands bf16 (fp32 PSUM accumulation); x-slab DMAs land first so
    the first projection matmul starts ~1.5us in; Scalar queue carries
    DMAs only before the first exp exists.
  - weave: prolog = chunk-0 V,K[t0],Q[t0]; chunk-0 attention weaves the
    remaining chunk-0 + all chunk-1 projections; chunk-1 attention weaves
    oproj m0-7 (chunk-0 rows) for h<4, then t0/t1-partials of m8-15
    (stashed to SBUF) for h>=4; tail = t2/t3 + stash-add only.
  - normalization: ones-column denominator in the A@V stationary operand,
    direct DVE reciprocal of the denominator row, gpsimd
    partition_broadcast, DVE multiply.
"""
import sys
import numpy as np

try:
    import concourse.bass as bass  # noqa: F401
except ImportError:  # pragma: no cover
    sys.path.insert(0, "/opt/trn_rl_repo")

import itertools
from contextlib import ExitStack

import concourse.bacc as bacc
import concourse.tile as tile
import concourse.mybir as mybir
import concourse.bass_utils as bass_utils

B, S, D, H = 4, 2048, 1024, 16
DK = D // H            # 64
G = 2                  # head groups (cores per batch)
HPG = H // G           # 8 heads per core
DG = HPG * DK          # 512 local head dims
NCORES = B * G         # 8
SC = 1024              # q-chunk width
NQC = S // SC          # 2
NKT = S // 128         # 16 k-tiles

F32 = mybir.dt.float32
BF16 = mybir.dt.bfloat16
AF = mybir.ActivationFunctionType

LAST_RESULT = None     # BassKernelResults of the most recent run (for test.py)
_prog = None


def _build():
    nc = bacc.Bacc("TRN2", target_bir_lowering=False, debug=False,
                   num_devices=NCORES)
    xT = nc.dram_tensor("xT", [D, S], BF16, kind="ExternalInput").ap()
    wq = nc.dram_tensor("wq", [D, DG], BF16, kind="ExternalInput").ap()
    wk = nc.dram_tensor("wk", [D, DG], BF16, kind="ExternalInput").ap()
    wv = nc.dram_tensor("wv", [D, DG], BF16, kind="ExternalInput").ap()
    wo = nc.dram_tensor("wo", [DG, D], BF16, kind="ExternalInput").ap()
    bq = nc.dram_tensor("bq", [DG, 1], F32, kind="ExternalInput").ap()
    bv = nc.dram_tensor("bv", [1, DG], F32, kind="ExternalInput").ap()
    tri = nc.dram_tensor("tri", [128, 128], BF16, kind="ExternalInput").ap()
    out = nc.dram_tensor("out", [S, D], F32, kind="ExternalOutput").ap()

    with tile.TileContext(nc) as tc, ExitStack() as ctx:
        cpool = ctx.enter_context(tc.tile_pool(name="consts", bufs=1))
        qkpool = ctx.enter_context(tc.tile_pool(name="qk", bufs=1))
        vpool = ctx.enter_context(tc.tile_pool(name="vaug", bufs=1))
        vtpool = ctx.enter_context(tc.tile_pool(name="vt", bufs=1))
        wpool = ctx.enter_context(tc.tile_pool(name="wqkv", bufs=1))
        xpool = ctx.enter_context(tc.tile_pool(name="xs", bufs=4))
        atpool = ctx.enter_context(tc.tile_pool(name="at", bufs=4))
        pdpool = ctx.enter_context(tc.tile_pool(name="pd", bufs=2))
        scpool = ctx.enter_context(tc.tile_pool(name="scr", bufs=2))
        rpool = ctx.enter_context(tc.tile_pool(name="r0", bufs=2))
        tmpool = ctx.enter_context(tc.tile_pool(name="tmp", bufs=2))
        opool = ctx.enter_context(tc.tile_pool(name="ost", bufs=2))
        stpool = ctx.enter_context(tc.tile_pool(name="stash", bufs=16))
        # PSUM: scores 2x[128,1024] (banks 0-3), AV 1x[128,1024] (4-5),
        # projection/oproj accumulators 2x[128,512] (6-7)
        ppsc = ctx.enter_context(tc.tile_pool(name="ppsc", bufs=2, space="PSUM"))
        ppav = ctx.enter_context(tc.tile_pool(name="ppav", bufs=1, space="PSUM"))
        ppacc = ctx.enter_context(tc.tile_pool(name="ppacc", bufs=2, space="PSUM"))

        xTr = xT.rearrange("(c p) s -> p c s", p=128)
        wqr = wq.rearrange("(c p) n -> p c n", p=128)
        wkr = wk.rearrange("(c p) n -> p c n", p=128)
        wvr = wv.rearrange("(c p) n -> p c n", p=128)

        # startup: all x slabs first (the V prolog contracts over all 8),
        # wv in parallel on gpsimd, then wq/wk. Scalar-queue DMAs are
        # startup-only (ACT idles until the first exp ~35us in).
        xs0_0 = xpool.tile([128, 4, SC], BF16, tag="xs", name="xs0_0")
        xs1_0 = xpool.tile([128, 4, SC], BF16, tag="xs", name="xs1_0")
        wq_t = wpool.tile([128, 8, DG], BF16)
        wk_t = wpool.tile([128, 8, DG], BF16)
        wv_t = wpool.tile([128, 8, DG], BF16)
        # bv + its broadcast go first: the V-group unpack (first prolog
        # consumer) needs bvfull
        bv_t = cpool.tile([1, DG], F32)
        nc.gpsimd.dma_start(bv_t[:], bv)
        bvfull = cpool.tile([128, DG], F32)
        nc.gpsimd.partition_broadcast(bvfull[:], bv_t[:])
        for c in range(4):
            nc.sync.dma_start(xs0_0[:, c, :], xTr[:, c, 0:SC])
            nc.scalar.dma_start(xs1_0[:, c, :], xTr[:, 4 + c, 0:SC])
        for c in range(8):
            nc.gpsimd.dma_start(wv_t[:, c, :], wvr[:, c, :])
        for c in range(4):
            nc.sync.dma_start(wq_t[:, c, :], wqr[:, c, :])
            nc.scalar.dma_start(wq_t[:, 4 + c, :], wqr[:, 4 + c, :])
            nc.sync.dma_start(wk_t[:, c, :], wkr[:, c, :])
            nc.scalar.dma_start(wk_t[:, 4 + c, :], wkr[:, 4 + c, :])

        bq_t = cpool.tile([128, 4], F32)
        nc.sync.dma_start(bq_t[:], bq.rearrange("(t p) o -> p (t o)", p=128))
        tri_t = cpool.tile([128, 128], BF16)
        nc.sync.dma_start(tri_t[:], tri)

        qt = [qkpool.tile([128, S], BF16, name=f"qt{t}") for t in range(4)]
        kt_ = [qkpool.tile([128, S], BF16, name=f"kt{t}") for t in range(4)]
        vt = [vtpool.tile([128, S], BF16, name=f"vt{t}") for t in range(4)]
        va = [vpool.tile([128, HPG * 65], BF16, name=f"va{i}") for i in range(NKT)]
        for i in range(NKT):
            nc.vector.memset(
                va[i].rearrange("p (h c) -> p h c", c=65)[:, :, 64:65], 1.0)
        # wo only needed by the output projection (chunk-1 filler onward)
        wo_t = cpool.tile([128, 4, D], BF16)
        nc.gpsimd.dma_start(wo_t[:], wo.rearrange("(c p) n -> p c n", p=128))

        def qgroup(sc, t, half, xc):
            pq = ppacc.tile([128, 512], F32, tag="pacc", name=f"pq{sc}_{t}_{half}")
            for c in range(8):
                nc.tensor.matmul(pq[:], wq_t[:, c, t * 128:(t + 1) * 128],
                                 xc(c)[:, half * 512:(half + 1) * 512],
                                 start=(c == 0), stop=(c == 7))
                yield
            lo = sc * SC + half * 512
            nc.vector.tensor_scalar_add(qt[t][:, lo:lo + 512], pq[:],
                                        bq_t[:, t:t + 1])

        def kgroup(sc, t, half, xc):
            pk = ppacc.tile([128, 512], F32, tag="pacc", name=f"pk{sc}_{t}_{half}")
            for c in range(8):
                nc.tensor.matmul(pk[:], wk_t[:, c, t * 128:(t + 1) * 128],
                                 xc(c)[:, half * 512:(half + 1) * 512],
                                 start=(c == 0), stop=(c == 7))
                yield
            lo = sc * SC + half * 512
            # no K bias: it shifts logits by a per-q constant along k,
            # which softmax cancels exactly
            nc.vector.tensor_copy(kt_[t][:, lo:lo + 512], pk[:])

        def vgroup(sc, ms, xc):
            pv = ppacc.tile([128, 512], F32, tag="pacc", name=f"pv{sc}_{ms}")
            for c in range(8):
                nc.tensor.matmul(pv[:], xc(c)[:, ms * 128:(ms + 1) * 128],
                                 wv_t[:, c, :], start=(c == 0), stop=(c == 7))
                yield
            i = 8 * sc + ms
            nc.vector.tensor_add(
                va[i].rearrange("p (h c) -> p h c", c=65)[:, :, 0:64],
                pv[:].rearrange("p (h c) -> p h c", c=64),
                bvfull[:].rearrange("p (h c) -> p h c", c=64))
            yield

        def make_xc(sc, slabs=None):
            if slabs is not None:
                xs0, xs1 = slabs
            else:
                xs0 = xpool.tile([128, 4, SC], BF16, tag="xs", name=f"xs0_{sc}")
                xs1 = xpool.tile([128, 4, SC], BF16, tag="xs", name=f"xs1_{sc}")
                for c in range(4):
                    nc.sync.dma_start(xs0[:, c, :],
                                      xTr[:, c, sc * SC:(sc + 1) * SC])
                    nc.gpsimd.dma_start(xs1[:, c, :],
                                        xTr[:, 4 + c, sc * SC:(sc + 1) * SC])
            return lambda c: (xs0 if c < 4 else xs1)[:, c % 4, :]

        def proj_rest_gen(sc, xc):
            # Q/K for t=1..3 (t=0 and V are done in the prolog for sc=0)
            for t in range(1, 4):
                for half in range(2):
                    yield from kgroup(sc, t, half, xc)
                for half in range(2):
                    yield from qgroup(sc, t, half, xc)

        def proj_full_gen(sc):
            xc = make_xc(sc)
            for ms in range(8):
                yield from vgroup(sc, ms, xc)
            for half in range(2):
                yield from kgroup(sc, 0, half, xc)
            for half in range(2):
                yield from qgroup(sc, 0, half, xc)
            yield from proj_rest_gen(sc, xc)

        def oproj_g(m):
            ot = opool.tile([128, D], F32, tag="ost", name=f"ot{m}")
            for nh in range(2):
                pon = ppacc.tile([128, 512], F32, tag="pacc", name=f"pon{m}_{nh}")
                for t in range(4):
                    nc.tensor.matmul(pon[:], vt[t][:, m * 128:(m + 1) * 128],
                                     wo_t[:, t, nh * 512:(nh + 1) * 512],
                                     start=(t == 0), stop=(t == 3))
                    yield
                nc.vector.tensor_copy(ot[:, nh * 512:(nh + 1) * 512], pon[:])
                yield
            nc.sync.dma_start(out[m * 128:(m + 1) * 128, :], ot[:])

        stash = {}

        def stash_seg(t, first=False):
            # one t-tile's contribution to oproj rows m8-15, accumulated in
            # an SBUF stash so the tail only needs the last t-tile's matmul
            for m in range(8, 16):
                for nh in range(2):
                    pon = ppacc.tile([128, 512], F32, tag="pacc",
                                     name=f"pp{t}_{m}_{nh}")
                    nc.tensor.matmul(pon[:], vt[t][:, m * 128:(m + 1) * 128],
                                     wo_t[:, t, nh * 512:(nh + 1) * 512],
                                     start=True, stop=True)
                    yield
                    if first:
                        st = stpool.tile([128, 512], BF16, tag="st",
                                         name=f"st{m}_{nh}")
                        nc.vector.tensor_copy(st[:], pon[:])
                        stash[(m, nh)] = st
                    else:
                        st = stash[(m, nh)]
                        nc.vector.tensor_add(st[:], pon[:], st[:])
                    yield

        def oproj_tail_g(m):
            ot = opool.tile([128, D], F32, tag="ost", name=f"ot{m}")
            for nh in range(2):
                pon = ppacc.tile([128, 512], F32, tag="pacc", name=f"pt{m}_{nh}")
                nc.tensor.matmul(pon[:], vt[2][:, m * 128:(m + 1) * 128],
                                 wo_t[:, 2, nh * 512:(nh + 1) * 512],
                                 start=True, stop=True)
                nc.vector.tensor_add(ot[:, nh * 512:(nh + 1) * 512], pon[:],
                                     stash[(m, nh)][:])
            nc.sync.dma_start(out[m * 128:(m + 1) * 128, :], ot[:])

        def attn_head(h, qc, pull):
            t, po = h // 2, 64 * (h % 2)
            nkt = 8 * qc + 8
            pav = ppav.tile([128, SC], F32, tag="pav", name=f"pav{h}_{qc}")
            for kti in range(nkt):
                delta = max(kti * 128 - qc * SC, 0)
                pss = ppsc.tile([128, SC], F32, tag="ps", name=f"ps{h}_{qc}_{kti}")
                ks = kt_[t][po:po + 64, kti * 128:(kti + 1) * 128]
                q0 = qc * SC
                if delta < 512:
                    nc.tensor.matmul(pss[:, delta:512], ks,
                                     qt[t][po:po + 64, q0 + delta:q0 + 512],
                                     start=True, stop=True)
                h1lo = max(delta, 512)
                nc.tensor.matmul(pss[:, h1lo:SC], ks,
                                 qt[t][po:po + 64, q0 + h1lo:q0 + SC],
                                 start=True, stop=True)
                at = atpool.tile([128, SC], BF16, tag="at", name=f"at{h}_{qc}_{kti}")
                nc.scalar.activation(at[:, delta:SC], pss[:, delta:SC],
                                     AF.Exp, scale=0.125)
                if kti >= 8 * qc:  # diagonal 128x128 square of the band tile
                    sl = slice(delta, delta + 128)
                    nc.vector.tensor_mul(at[:, sl], at[:, sl], tri_t[:])
                vs = va[kti][:, h * 65:(h + 1) * 65]
                if delta < 512:
                    nc.tensor.matmul(pav[0:65, delta:512], vs, at[:, delta:512],
                                     start=(kti == 0), stop=(kti == 8 * qc + 3))
                nc.tensor.matmul(pav[0:65, h1lo:SC], vs, at[:, h1lo:SC],
                                 start=(kti == 0), stop=(kti == nkt - 1))
                pull(4 if qc == 0 else 2)
            # drain + normalize: denominator sits in row 64 of pav
            pd = pdpool.tile([128, SC], F32, tag="pd", name=f"pd{h}_{qc}")
            nc.vector.tensor_copy(pd[0:65, :], pav[0:65, :])
            # spread the 1024 denominators over 128 partitions: the exact
            # reciprocal runs at 8 elements/lane instead of 1024 on one lane
            rb = rpool.tile([128, 8], F32, tag="rb", name=f"rb{h}_{qc}")
            nc.sync.dma_start(rb[:], pd[64:65, :])
            nc.vector.reciprocal(rb[:], rb[:])
            r0 = rpool.tile([1, SC], F32, tag="r0", name=f"r0{h}_{qc}")
            nc.sync.dma_start(r0[:], rb[:])
            sct = scpool.tile([128, SC], F32, tag="scr", name=f"sc{h}_{qc}")
            nc.gpsimd.partition_broadcast(sct[0:64, :], r0[:])
            if po == 0:
                nc.vector.tensor_mul(vt[t][0:64, qc * SC:(qc + 1) * SC],
                                     pd[0:64, :], sct[0:64, :])
            else:
                tmp = tmpool.tile([64, SC], BF16, tag="tmp", name=f"tm{h}_{qc}")
                nc.vector.tensor_mul(tmp[:], pd[0:64, :], sct[0:64, :])
                nc.sync.dma_start(vt[t][64:128, qc * SC:(qc + 1) * SC], tmp[:])
            pull(2)

        # ---- prolog: chunk-0 V, K[t0], Q[t0] (minimum for attention h=0)
        xc0 = make_xc(0, slabs=(xs0_0, xs1_0))
        for ms in range(8):
            for _ in vgroup(0, ms, xc0):
                pass
        for half in range(2):
            for _ in kgroup(0, 0, half, xc0):
                pass
        for half in range(2):
            for _ in qgroup(0, 0, half, xc0):
                pass

        # ---- chunk 0: attention, weaving the rest of proj-0 then proj-1
        filler = itertools.chain(proj_rest_gen(0, xc0), proj_full_gen(1))

        def pull(n):
            for _ in range(n):
                if next(filler, "END") == "END":
                    return

        for h in range(HPG):
            attn_head(h, 0, pull)
        for _ in filler:   # any proj-1 leftovers before chunk-1 attention
            pass

        # ---- chunk 1: heads ordered so the t2 heads (4,5) come last; the
        # filler gains each t-tile's m8-15 stash segment two positions after
        # its heads complete (one-position margin over the normalize chain).
        # t3-partial leftovers drain after the loop, overlapping the last
        # normalize chain; the tail then only needs the 16 t2 matmuls.
        filler = (y for m in range(8) for y in oproj_g(m))
        for pos, h in enumerate([0, 1, 2, 3, 6, 7, 4, 5]):
            if pos == 3:
                filler = itertools.chain(filler, stash_seg(0, first=True))
            elif pos == 5:
                filler = itertools.chain(filler, stash_seg(1))
            elif pos == 7:
                filler = itertools.chain(filler, stash_seg(3))
            cap = 2 if pos < 7 else 1

            def pull(n, _f=filler, _cap=cap):
                for _ in range(min(n, _cap)):
                    if next(_f, "END") == "END":
                        return
            attn_head(h, 1, pull)
        for _ in filler:   # t3 partials overlap the last normalize chain
            pass
        # ---- tail: finish m8-15 with the t2 matmul + stash add
        for m in range(8, 16):
            oproj_tail_g(m)

    nc.compile()
    return nc


def _program():
    global _prog
    if _prog is None:
        _prog = _build()
    return _prog


def kernel(x, mask, Wq, bq, Wk, bk, Wv, bv, Wo, bo):
    global LAST_RESULT
    import ml_dtypes
    x = np.asarray(x, dtype=np.float32)
    Wq = np.asarray(Wq, dtype=np.float32)
    Wk = np.asarray(Wk, dtype=np.float32)
    Wv = np.asarray(Wv, dtype=np.float32)
    Wo = np.asarray(Wo, dtype=np.float32)
    bq = np.asarray(bq, dtype=np.float32)
    bv = np.asarray(bv, dtype=np.float32)
    bo = np.asarray(bo, dtype=np.float32)

    nc = _program()
    bf = ml_dtypes.bfloat16
    xTs = [np.ascontiguousarray(x[b].T).astype(bf) for b in range(B)]
    tri = np.zeros((128, 128), dtype=bf)
    tri[np.triu_indices(128)] = 1.0
    in_maps = []
    for c in range(NCORES):
        b, g = divmod(c, G)
        sl = slice(g * DG, (g + 1) * DG)
        in_maps.append({
            "xT": xTs[b],
            "wq": np.ascontiguousarray(Wq[sl, :].T).astype(bf),
            "wk": np.ascontiguousarray(Wk[sl, :].T).astype(bf),
            "wv": np.ascontiguousarray(Wv[sl, :].T).astype(bf),
            "wo": np.ascontiguousarray(Wo[:, sl].T).astype(bf),
            "bq": np.ascontiguousarray(bq[sl].reshape(DG, 1)),
            "bv": np.ascontiguousarray(bv[sl].reshape(1, DG)),
            "tri": tri,
        })
    res = bass_utils.run_bass_kernel_spmd(nc, in_maps,
                                          core_ids=list(range(NCORES)))
    LAST_RESULT = res
    outs = [r["out"] for r in res.results]
    y = np.stack([outs[G * b] + outs[G * b + 1] for b in range(B)])
    y += bo[None, None, :]
    return y.astype(np.float32)
